# revision 18
# baseline (speedup 1.0000x reference)
"""CosineSelfAttention (linear attention) TRN2 Bass kernel.

Sharding: 8 cores = 4 batches x 2 head-groups (8 heads each). Each core
computes its batch's projections for its 512 output columns, the per-head
kv/ksum/vsum reductions, and the normalized context. No collectives.

All matmuls run as float32r (full-rate on PE at N>=256, ~1.2e-4 rounding).
hs is passed pre-transposed ([DM, S]) so no on-device hs transposes needed.
"""

import numpy as np
import bass_rust
import concourse.bass as bass
import concourse.mybir as mybir
import concourse.tile as tile
from concourse.bass_utils import run_bass_kernel_spmd
import concourse.tile_sem_assignment as _tsa

# All HWDGE DMAs on one sem lane: SP-ring FIFO keeps threshold ordering
# sound, and merged thresholds keep matmul sync-waits at 1 (HW limit).
_tsa.NUM_HWDGE_SEMS = 1

F32 = mybir.dt.float32
F32R = mybir.dt.float32r
ADD = mybir.AluOpType.add
MULT = mybir.AluOpType.mult

B, S, DM, H, D = 4, 4096, 1024, 16, 64
EPS = 1e-5
MC = 512          # per-core output columns (8 heads x 64)
NT = 32           # s-tiles of 128 tokens
NCH = 8           # chunks of 512 tokens
NDM = 8           # dm-tiles of 128


def _build():
    nc = bass.Bass()
    wc_sem = nc.alloc_semaphore("wc_sem")

    hsT = nc.declare_dram_parameter("hsT", [DM, S], F32R, isOutput=False)
    wq = nc.declare_dram_parameter("wq", [DM, MC], F32R, isOutput=False)
    wk = nc.declare_dram_parameter("wk", [DM, MC], F32R, isOutput=False)
    wv = nc.declare_dram_parameter("wv", [DM, MC], F32R, isOutput=False)
    identr = nc.declare_dram_parameter("identr", [128, 128], F32R, isOutput=False)
    onesc = nc.declare_dram_parameter("onesc", [128, 1], F32R, isOutput=False)
    onesr = nc.declare_dram_parameter("onesr", [1, 128], F32R, isOutput=False)
    bq = nc.declare_dram_parameter("bq", [128, MC], F32, isOutput=False)
    bk = nc.declare_dram_parameter("bk", [128, MC], F32, isOutput=False)
    bv = nc.declare_dram_parameter("bv", [128, MC], F32, isOutput=False)
    mvec = nc.declare_dram_parameter("mvec", [128, NT], F32, isOutput=False)
    cvec = nc.declare_dram_parameter("cvec", [128, 1], F32, isOutput=False)
    zer8 = nc.declare_dram_parameter("zer8", [128, 8], F32R, isOutput=False)
    out = nc.declare_dram_parameter("out", [S, MC], F32, isOutput=True)

    with tile.TileContext(nc) as tc:
      with tc.tile_pool(name="const", bufs=1) as cp, \
           tc.tile_pool(name="io", bufs=2) as iop, \
           tc.tile_pool(name="wk1", bufs=2) as wkp, \
           tc.tile_pool(name="jnk", space="PSUM", bufs=1) as jpp:
        # ---- constants / weights ----
        wq_sb = cp.tile([128, NDM * MC], F32R, tag="wq_sb")
        wk_sb = cp.tile([128, NDM * MC], F32R, tag="wk_sb")
        wv_sb = cp.tile([128, NDM * MC], F32R, tag="wv_sb")
        nc.sync.dma_start(out=wq_sb[:].rearrange("p (j m) -> p j m", j=NDM), in_=wq[:].rearrange("(j p) m -> p j m", p=128))
        nc.sync.dma_start(out=wk_sb[:].rearrange("p (j m) -> p j m", j=NDM), in_=wk[:].rearrange("(j p) m -> p j m", p=128))
        nc.sync.dma_start(out=wv_sb[:].rearrange("p (j m) -> p j m", j=NDM), in_=wv[:].rearrange("(j p) m -> p j m", p=128))
        id_sb = cp.tile([128, 128], F32R, tag="id_sb")
        nc.sync.dma_start(out=id_sb[:], in_=identr[:])
        onesc_sb = cp.tile([128, 1], F32R, tag="onesc_sb")
        nc.sync.dma_start(out=onesc_sb[:], in_=onesc[:])
        onesr_sb = cp.tile([1, 128], F32R, tag="onesr_sb")
        nc.sync.dma_start(out=onesr_sb[:], in_=onesr[:])
        bq_sb = cp.tile([128, MC], F32, tag="bq_sb")
        bk_sb = cp.tile([128, MC], F32, tag="bk_sb")
        bv_sb = cp.tile([128, MC], F32, tag="bv_sb")
        nc.sync.dma_start(out=bq_sb[:], in_=bq[:])
        nc.sync.dma_start(out=bk_sb[:], in_=bk[:])
        nc.sync.dma_start(out=bv_sb[:], in_=bv[:])
        m_sb = cp.tile([128, NT], F32, tag="m_sb")
        nc.sync.dma_start(out=m_sb[:], in_=mvec[:])
        cv_sb = cp.tile([128, 1], F32, tag="cv_sb")
        nc.sync.dma_start(out=cv_sb[:], in_=cvec[:])

        # persistent intermediates
        qT_sb = [cp.tile([128, S], F32R, tag=f"qT{j}", name=f"qT{j}") for j in range(4)]
        rq_all = cp.tile([128, 8 * NT], F32, tag="rq_all")
        kv_sb = cp.tile([128, 256], F32R, tag="kv_sb")
        ksum_sb = cp.tile([1, MC], F32R, tag="ksum_sb")
        vsum_sb = cp.tile([1, MC], F32R, tag="vsum_sb")
        ksumT_sb = cp.tile([128, 4], F32R, tag="ksumT_sb")
        ksumT3 = cp.tile([128, 8], F32R, tag="ksumT3")
        hssum_sb = cp.tile([128, NDM], F32, tag="hssum_sb")
        hssum_r = cp.tile([128, NDM], F32R, tag="hssum_r")
        ksum_bc = cp.tile([128, MC], F32, tag="ksum_bc")
        vsum_bc = cp.tile([128, MC], F32, tag="vsum_bc")

        # junk templates for cap_waits
        junk_ps = jpp.tile([1, 2], F32, tag="junk", bufs=1)
        nc.tensor.matmul(junk_ps[:], id_sb[:, 0:1], id_sb[:, 0:2], start=True, stop=True)
        junk_sb = cp.tile([1, 8], F32, tag="junk_sb")
        nc.sync.dma_start(out=junk_sb[0:1, 2:3], in_=junk_sb[0:1, 0:1])
        nc.vector.memset(junk_sb[0:1, 4:5], 0.0)
        nc.scalar.copy(junk_sb[0:1, 6:7], junk_sb[0:1, 4:5])
        nc.gpsimd.memset(junk_sb[0:1, 7:8], 0.0)

        # ======== PASS 1: projections, k-normalize, reductions, qT ========
        with tc.tile_pool(name="ps1", space="PSUM", bufs=1) as ps1:
            kv_ps = ps1.tile([128, MC], F32, tag="kv", bufs=1)
            ksum_ps = ps1.tile([1, MC], F32, tag="ksum", bufs=1)

            for ch in range(NCH):
                hsT_c = iop.tile([128, NDM * 512], F32R, tag="hsT_c")
                nc.sync.dma_start(
                    out=hsT_c[:].rearrange("p (j t) -> p j t", j=NDM),
                    in_=hsT[:, ch * 512:(ch + 1) * 512].rearrange(
                        "(j p) t -> p j t", p=128),
                )
                hs_part = wkp.tile([128, NDM], F32, tag="hs_part", bufs=2)
                nc.vector.tensor_reduce(
                    hs_part[:], hsT_c[:].rearrange("p (j t) -> p j t", j=NDM),
                    axis=mybir.AxisListType.X, op=ADD)
                if ch == 0:
                    nc.vector.tensor_copy(hssum_sb[:], hs_part[:])
                else:
                    nc.vector.tensor_tensor(hssum_sb[:], hssum_sb[:], hs_part[:], ADD)
                for j4 in range(4):
                    si = ch * 4 + j4
                    pq = ps1.tile([128, MC], F32, tag="proj", bufs=3)
                    pk = ps1.tile([128, MC], F32, tag="proj", bufs=3)
                    pv = ps1.tile([128, MC], F32, tag="proj", bufs=3)
                    for dm in range(NDM):
                        blk = hsT_c[:, dm * 512 + j4 * 128: dm * 512 + (j4 + 1) * 128]
                        nc.tensor.matmul(pq[:], blk, wq_sb[:, dm * MC:(dm + 1) * MC],
                                         start=(dm == 0), stop=(dm == NDM - 1))
                        nc.tensor.matmul(pk[:], blk, wk_sb[:, dm * MC:(dm + 1) * MC],
                                         start=(dm == 0), stop=(dm == NDM - 1))
                        nc.tensor.matmul(pv[:], blk, wv_sb[:, dm * MC:(dm + 1) * MC],
                                         start=(dm == 0), stop=(dm == NDM - 1))

                    m_ap = m_sb[:, si:si + 1]
                    # q = (pq + bq) * m   -> F32R
                    scr_q = wkp.tile([128, MC], F32, tag="scr_q", bufs=1)
                    nc.vector.tensor_tensor(scr_q[:], pq[:], bq_sb[:], ADD)
                    q_r = wkp.tile([128, MC], F32R, tag="q_r")
                    nc.scalar.activation(q_r[:], scr_q[:],
                                         mybir.ActivationFunctionType.Copy,
                                         scale=m_ap)
                    # k = (pk + bk) * m   -> F32R
                    scr_k = wkp.tile([128, MC], F32, tag="scr_k", bufs=1)
                    nc.vector.tensor_tensor(scr_k[:], pk[:], bk_sb[:], ADD)
                    k_r = wkp.tile([128, MC], F32R, tag="k_r", bufs=1)
                    nc.scalar.activation(k_r[:], scr_k[:],
                                         mybir.ActivationFunctionType.Copy,
                                         scale=m_ap)
                    # v = pv + bv  -> F32R
                    v_r = wkp.tile([128, MC], F32R, tag="v_r")
                    nc.vector.tensor_tensor(v_r[:], pv[:], bv_sb[:], ADD)

                    # rq = 1/(||q|| + EPS) per (token, head)
                    sq = wkp.tile([128, MC], F32, tag="sq", bufs=1)
                    nc.scalar.square(sq[:], q_r[:])
                    s2 = wkp.tile([128, 8], F32, tag="s2")
                    nc.vector.tensor_reduce(
                        s2[:], sq[:].rearrange("p (g d) -> p g d", g=8),
                        axis=mybir.AxisListType.X, op=ADD)
                    nn_ = wkp.tile([128, 8], F32, tag="nn_")
                    nc.scalar.sqrt(nn_[:], s2[:])
                    nc.vector.tensor_scalar_add(nn_[:], nn_[:], EPS)
                    nc.vector.reciprocal(rq_all[:, si * 8:(si + 1) * 8], nn_[:])

                    # k-hat = k / (||k|| + EPS)
                    sqk = wkp.tile([128, MC], F32, tag="sq", bufs=1)
                    nc.scalar.square(sqk[:], k_r[:])
                    s2k = wkp.tile([128, 8], F32, tag="s2")
                    nc.vector.tensor_reduce(
                        s2k[:], sqk[:].rearrange("p (g d) -> p g d", g=8),
                        axis=mybir.AxisListType.X, op=ADD)
                    nk = wkp.tile([128, 8], F32, tag="nn_")
                    nc.scalar.sqrt(nk[:], s2k[:])
                    nc.vector.tensor_scalar_add(nk[:], nk[:], EPS)
                    rg = wkp.tile([128, 8], F32, tag="rg")
                    nc.vector.reciprocal(rg[:], nk[:])
                    kh_r = wkp.tile([128, MC], F32R, tag="kh_r")
                    nc.vector.tensor_tensor(
                        kh_r[:].rearrange("p (g d) -> p g d", g=8),
                        k_r[:].rearrange("p (g d) -> p g d", g=8),
                        rg[:].unsqueeze(2).broadcast_to([128, 8, 64]), MULT)

                    # kv accumulation: per pair p, [128(2h), 128(2h e)] block
                    for p in range(4):
                        nc.tensor.matmul(
                            kv_ps[:, p * 128:(p + 1) * 128],
                            kh_r[:, p * 128:(p + 1) * 128],
                            v_r[:, p * 128:(p + 1) * 128],
                            start=(si == 0 and p == 0),
                            stop=(si == NT - 1 and p == 3),
                            skip_group_check=True)
                    # ksum / vsum
                    nc.tensor.matmul(ksum_ps[:], onesc_sb[:], kh_r[:],
                                     start=(si == 0), stop=(si == NT - 1),
                                     skip_group_check=True)

                    # qT via PE transpose
                    for j in range(4):
                        tq = ps1.tile([128, 128], F32R, tag="tq", bufs=2)
                        nc.tensor.transpose(tq[:], q_r[:, j * 128:(j + 1) * 128], id_sb[:])
                        nc.scalar.copy(
                            qT_sb[j][:, si * 128:(si + 1) * 128], tq[:])

            # ---- epilogue of pass 1 ----
            # kv diagonal blocks -> kv_sb [128, 256]
            for p in range(4):
                nc.vector.tensor_copy(
                    kv_sb[0:64, p * 64:(p + 1) * 64],
                    kv_ps[0:64, p * 128:p * 128 + 64])
                nc.vector.tensor_copy(
                    kv_sb[64:128, p * 64:(p + 1) * 64],
                    kv_ps[64:128, p * 128 + 64:(p + 1) * 128])
            nc.vector.tensor_copy(ksum_sb[:], ksum_ps[:])
            nc.vector.tensor_copy(hssum_r[:], hssum_sb[:])
            vsum_ps = ps1.tile([1, MC], F32, tag="proj", bufs=3)
            for dm in range(NDM):
                nc.tensor.matmul(vsum_ps[:], hssum_r[:, dm:dm + 1],
                                 wv_sb[:, dm * MC:(dm + 1) * MC],
                                 start=(dm == 0), stop=(dm == NDM - 1))
            nc.vector.tensor_copy(vsum_sb[:], vsum_ps[:])
            # ksumT [128, 4] via tiny transposes (N=2 with junk col)
            for j in range(4):
                tk = ps1.tile([128, 2], F32R, tag="tq", bufs=2)
                nc.tensor.transpose(
                    tk[:], ksum_sb[0:1, j * 128:(j + 1) * 128], id_sb[0:1, 0:2])
                nc.vector.tensor_copy(ksumT_sb[:, j:j + 1], tk[:, 0:1])
            # ksumT3 [128, 8]: per pair p, col 2p = ksum_{2p} on rows 0-63,
            # col 2p+1 = ksum_{2p+1} on rows 64-127, zeros elsewhere
            nc.sync.dma_start(out=ksumT3[:], in_=zer8[:])
            for p in range(4):
                nc.vector.tensor_copy(ksumT3[0:64, 2 * p:2 * p + 1],
                                      ksumT_sb[0:64, p:p + 1])
                nc.vector.tensor_copy(ksumT3[64:128, 2 * p + 1:2 * p + 2],
                                      ksumT_sb[64:128, p:p + 1])
            # partition-broadcast of ksum / vsum via K=1 matmuls
            pbc = ps1.tile([128, MC], F32, tag="proj", bufs=3)
            nc.tensor.matmul(pbc[:], onesr_sb[:], ksum_sb[:], start=True, stop=True)
            nc.vector.tensor_copy(ksum_bc[:], pbc[:])
            pbc2 = ps1.tile([128, MC], F32, tag="proj", bufs=3)
            nc.tensor.matmul(pbc2[:], onesr_sb[:], vsum_sb[:], start=True, stop=True)
            nc.vector.scalar_tensor_tensor(vsum_bc[:], bv_sb[:], float(S), pbc2[:],
                                           MULT, ADD)

        # ======== PASS 2: ctxT = kv^T qT, u, re-transpose, normalize ========
        with tc.tile_pool(name="ps2", space="PSUM", bufs=1) as ps2:
            for ch in range(NCH):
                cs = slice(ch * 512, (ch + 1) * 512)
                ctx_nat = [wkp.tile([128, MC], F32, tag=f"cn{j}", bufs=1, name=f"cn{j}")
                           for j in range(4)]
                u_nat = [wkp.tile([128, 8], F32, tag=f"un{j}", bufs=2, name=f"un{j}")
                         for j in range(4)]
                for p in range(4):
                    psA = ps2.tile([64, MC], F32, tag="ct", bufs=3)
                    psB = ps2.tile([64, MC], F32, tag="ct", bufs=3)
                    nc.tensor.matmul(psA[:], kv_sb[0:64, p * 64:(p + 1) * 64],
                                     qT_sb[p][0:64, cs], start=True, stop=True,
                                     tile_position=(0, 0))
                    nc.tensor.matmul(psB[:], kv_sb[64:128, p * 64:(p + 1) * 64],
                                     qT_sb[p][64:128, cs], start=True, stop=True,
                                     tile_position=(64, 0))
                    for j in range(4):
                        si = ch * 4 + j
                        pu = ps2.tile([128, 2], F32, tag="u", bufs=2)
                        nc.tensor.matmul(
                            pu[:], qT_sb[p][:, si * 128:(si + 1) * 128],
                            ksumT3[:, 2 * p:2 * p + 2], start=True, stop=True)
                        nc.vector.tensor_copy(u_nat[j][:, 2 * p:2 * p + 2], pu[:])
                    ctxT_sb = wkp.tile([128, MC], F32R, tag="ctxT_sb")
                    nc.scalar.copy(ctxT_sb[0:64, :], psA[:])
                    nc.scalar.copy(ctxT_sb[64:128, :], psB[:])
                    for j in range(4):
                        rt = ps2.tile([128, 128], F32R, tag="rt", bufs=2)
                        nc.tensor.transpose(
                            rt[:], ctxT_sb[:, j * 128:(j + 1) * 128], id_sb[:])
                        nc.scalar.copy(
                            ctx_nat[j][:, p * 128:(p + 1) * 128], rt[:])
                # normalize + store
                o_ch = wkp.tile([128, 4 * MC], F32, tag="o_ch", bufs=1)
                for j in range(4):
                    si = ch * 4 + j
                    rq = rq_all[:, si * 8:(si + 1) * 8]
                    onum = wkp.tile([128, MC], F32, tag="onum", bufs=1)
                    nc.vector.tensor_tensor(
                        onum[:].rearrange("p (g d) -> p g d", g=8),
                        ctx_nat[j][:].rearrange("p (g d) -> p g d", g=8),
                        rq.unsqueeze(2).broadcast_to([128, 8, 64]), MULT)
                    nc.gpsimd.tensor_tensor(onum[:], onum[:], vsum_bc[:], ADD)
                    dn = wkp.tile([128, 8], F32, tag="dn")
                    nc.vector.tensor_tensor(dn[:], u_nat[j][:], rq, MULT)
                    nc.vector.tensor_scalar_add(dn[:], dn[:], cv_sb[:, 0:1])
                    dn2 = wkp.tile([128, 8], F32, tag="dn2")
                    nc.vector.reciprocal(dn2[:], dn[:])
                    nc.vector.tensor_tensor(
                        o_ch[:, j * MC:(j + 1) * MC].rearrange("p (g d) -> p g d", g=8),
                        onum[:].rearrange("p (g d) -> p g d", g=8),
                        dn2[:].unsqueeze(2).broadcast_to([128, 8, 64]), MULT)
                nc.sync.dma_start(
                    out=out[ch * 512:(ch + 1) * 512, :].rearrange(
                        "(j p) m -> p j m", p=128),
                    in_=o_ch[:].rearrange("p (j m) -> p j m", j=4))

    _cap_waits(nc, wc_sem)
    return nc


def _cap_waits(nc, wc_sem):
    """trn2 walrus codegen allows only 1 sync wait per engine instruction.
    Move excess waits onto cloned junk instructions (same engine) inserted
    immediately before, each updating a dedicated junk semaphore."""
    f = nc.m.functions[0]
    tmpl = {}
    for b in f.blocks:
        for i in b.instructions:
            tn = type(i).__name__
            if tn == "InstMatmult" and "PE" not in tmpl and "junk" in str(i.outs[0]):
                tmpl["PE"] = i
            elif tn == "InstDMACopy" and "DMA" not in tmpl and "junk_sb" in str(i.outs[0]):
                tmpl["DMA"] = i
            elif tn == "InstMemset" and "junk_sb" in str(i.outs[0]):
                if i.engine == mybir.EngineType.DVE and "DVE" not in tmpl:
                    tmpl["DVE"] = i
                elif i.engine == mybir.EngineType.Pool and "POOL" not in tmpl:
                    tmpl["POOL"] = i
            elif tn == "InstActivation" and "ACT" not in tmpl and "junk_sb" in str(i.outs[0]):
                tmpl["ACT"] = i
    n = 0

    def make_carrier(kind, w):
        nonlocal n
        n += 1
        t = tmpl[kind]
        if kind == "PE":
            c = mybir.InstMatmult(
                name=f"Iwc-{n}",
                replication_resolution=0, replication_shift_amnt=0,
                replication_num_rows=0,
                start_tensor_calc=True, stop_tensor_calc=True,
                ins=list(t.ins), outs=list(t.outs),
                tile_size=t.tile_size, tile_position=t.tile_position,
            )
        elif kind == "DMA":
            c = mybir.InstDMACopy(
                name=f"Iwc-{n}", queue=t.queue, mode="Copy",
                ins=list(t.ins), outs=list(t.outs),
            )
        elif kind in ("DVE", "POOL"):
            c = mybir.InstMemset(
                name=f"Iwc-{n}", mode=t.mode, ins=[], outs=list(t.outs), constant=0,
            )
        else:
            c = mybir.InstActivation(
                name=f"Iwc-{n}", func=t.func, ins=list(t.ins), outs=list(t.outs),
            )
        c.engine = t.engine
        upd = bass_rust.SyncUpdate(
            sync_type="semaphore", id=wc_sem.num, ant_name=wc_sem.name,
            update_mode="sem-inc", update_value=1,
        )
        c.sync_info = bass_rust.SyncInfo(on_wait=[w], on_update=[upd])
        return c

    def kind_of(ins):
        eng = ins.engine
        if eng == mybir.EngineType.PE:
            return "PE"
        if eng == mybir.EngineType.SP:
            return "DMA"
        if eng == mybir.EngineType.DVE:
            return "DVE"
        if eng == mybir.EngineType.Activation:
            return "ACT"
        if eng == mybir.EngineType.Pool:
            return "POOL"
        return None

    for b in f.blocks:
        insts = b.instructions
        k = 0
        while k < len(insts):
            ins = insts[k]
            if not ins.name.startswith("Iwc"):
                si = ins.sync_info
                if si is not None and len(si.on_wait) > 1:
                    kind = kind_of(ins)
                    if kind is not None and kind in tmpl:
                        waits = list(si.on_wait)
                        for j, w in enumerate(waits[:-1]):
                            insts.insert(k + j, make_carrier(kind, w))
                        k += len(waits) - 1
                        ins.sync_info.on_wait = waits[-1:]
            k += 1


_CACHE = {}


def _get_nc():
    if "nc" not in _CACHE:
        _CACHE["nc"] = _build()
    return _CACHE["nc"]


def kernel(**inputs):
    hs = np.ascontiguousarray(np.asarray(inputs["hidden_states"], dtype=np.float32))
    mask = np.asarray(inputs["attention_mask"], dtype=np.float32)
    Wq = np.asarray(inputs["Wq"], dtype=np.float32)
    Wk = np.asarray(inputs["Wk"], dtype=np.float32)
    Wv = np.asarray(inputs["Wv"], dtype=np.float32)
    bq = np.asarray(inputs["bq"], dtype=np.float32)
    bk = np.asarray(inputs["bk"], dtype=np.float32)
    bv = np.asarray(inputs["bv"], dtype=np.float32)

    nc = _get_nc()
    ident = np.eye(128, dtype=np.float32)
    onesc = np.ones((128, 1), dtype=np.float32)
    onesr = np.ones((1, 128), dtype=np.float32)

    in_maps = []
    for core in range(8):
        b = core // 2
        hg = core % 2
        cols = slice(hg * MC, (hg + 1) * MC)
        m_s = mask[b, 0, 0, :] / 10000.0 + 1.0           # [S]
        c_const = EPS + float(m_s.sum())
        in_maps.append({
            "hsT": np.ascontiguousarray(hs[b].T),
            "wq": np.ascontiguousarray(Wq[:, cols]),
            "wk": np.ascontiguousarray(Wk[:, cols]),
            "wv": np.ascontiguousarray(Wv[:, cols]),
            "identr": ident,
            "onesc": onesc,
            "onesr": onesr,
            "bq": np.broadcast_to(bq[cols], (128, MC)).copy(),
            "bk": np.broadcast_to(bk[cols], (128, MC)).copy(),
            "bv": np.broadcast_to(bv[cols], (128, MC)).copy(),
            "mvec": np.ascontiguousarray(m_s.reshape(NT, 128).T),
            "cvec": np.full((128, 1), c_const, dtype=np.float32),
            "zer8": np.zeros((128, 8), dtype=np.float32),
        })

    res = run_bass_kernel_spmd(nc, in_maps, list(range(8)))
    out_full = np.empty((B, S, DM), dtype=np.float32)
    for core in range(8):
        b = core // 2
        hg = core % 2
        out_full[b, :, hg * MC:(hg + 1) * MC] = res.results[core]["out"]
    return out_full


# revision 19
# speedup vs baseline: 1.0659x; 1.0659x over previous
"""CosineSelfAttention (linear attention) TRN2 Bass kernel.

Sharding: 8 cores = 4 batches x 2 head-groups (8 heads each). Each core
computes its batch's projections for its 512 output columns, the per-head
kv/ksum/vsum reductions, and the normalized context. No collectives.

All matmuls run as float32r (full-rate on PE at N>=256, ~1.2e-4 rounding).
hs is passed pre-transposed ([DM, S]) so no on-device hs transposes needed.
"""

import numpy as np
import bass_rust
import concourse.bass as bass
import concourse.mybir as mybir
import concourse.tile as tile
from concourse.bass_utils import run_bass_kernel_spmd
import concourse.tile_sem_assignment as _tsa

# All HWDGE DMAs on one sem lane: SP-ring FIFO keeps threshold ordering
# sound, and merged thresholds keep matmul sync-waits at 1 (HW limit).
_tsa.NUM_HWDGE_SEMS = 1

F32 = mybir.dt.float32
F32R = mybir.dt.float32r
ADD = mybir.AluOpType.add
MULT = mybir.AluOpType.mult

B, S, DM, H, D = 4, 4096, 1024, 16, 64
EPS = 1e-5
MC = 512          # per-core output columns (8 heads x 64)
NT = 32           # s-tiles of 128 tokens
NCH = 8           # chunks of 512 tokens
NDM = 8           # dm-tiles of 128


def _build():
    nc = bass.Bass()
    wc_sem = nc.alloc_semaphore("wc_sem")

    hsT = nc.declare_dram_parameter("hsT", [DM, S], F32R, isOutput=False)
    wq = nc.declare_dram_parameter("wq", [DM, MC], F32R, isOutput=False)
    wk = nc.declare_dram_parameter("wk", [DM, MC], F32R, isOutput=False)
    wv = nc.declare_dram_parameter("wv", [DM, MC], F32R, isOutput=False)
    identr = nc.declare_dram_parameter("identr", [128, 128], F32R, isOutput=False)
    onesc = nc.declare_dram_parameter("onesc", [128, 1], F32R, isOutput=False)
    onesr = nc.declare_dram_parameter("onesr", [1, 128], F32R, isOutput=False)
    bq = nc.declare_dram_parameter("bq", [128, MC], F32, isOutput=False)
    bk = nc.declare_dram_parameter("bk", [128, MC], F32, isOutput=False)
    bv = nc.declare_dram_parameter("bv", [128, MC], F32, isOutput=False)
    mvec = nc.declare_dram_parameter("mvec", [128, NT], F32, isOutput=False)
    cvec = nc.declare_dram_parameter("cvec", [128, 1], F32, isOutput=False)
    zer8 = nc.declare_dram_parameter("zer8", [128, 8], F32R, isOutput=False)
    out = nc.declare_dram_parameter("out", [S, MC], F32, isOutput=True)

    with tile.TileContext(nc) as tc:
      with tc.tile_pool(name="const", bufs=1) as cp, \
           tc.tile_pool(name="io", bufs=2) as iop, \
           tc.tile_pool(name="wk1", bufs=2) as wkp, \
           tc.tile_pool(name="jnk", space="PSUM", bufs=1) as jpp:
        # ---- chunk-0 activations first: first projections need hsT0+W ----
        hsT_c0 = iop.tile([128, NDM * 512], F32R, tag="hsT_c", name="hsT_c0")
        nc.sync.dma_start(
            out=hsT_c0[:].rearrange("p (j t) -> p j t", j=NDM),
            in_=hsT[:, 0:512].rearrange("(j p) t -> p j t", p=128))
        # ---- constants / weights ----
        wq_sb = cp.tile([128, NDM * MC], F32R, tag="wq_sb")
        wk_sb = cp.tile([128, NDM * MC], F32R, tag="wk_sb")
        wv_sb = cp.tile([128, NDM * MC], F32R, tag="wv_sb")
        nc.sync.dma_start(out=wq_sb[:].rearrange("p (j m) -> p j m", j=NDM), in_=wq[:].rearrange("(j p) m -> p j m", p=128))
        nc.sync.dma_start(out=wk_sb[:].rearrange("p (j m) -> p j m", j=NDM), in_=wk[:].rearrange("(j p) m -> p j m", p=128))
        nc.sync.dma_start(out=wv_sb[:].rearrange("p (j m) -> p j m", j=NDM), in_=wv[:].rearrange("(j p) m -> p j m", p=128))
        id_sb = cp.tile([128, 128], F32R, tag="id_sb")
        nc.sync.dma_start(out=id_sb[:], in_=identr[:])
        onesc_sb = cp.tile([128, 1], F32R, tag="onesc_sb")
        nc.sync.dma_start(out=onesc_sb[:], in_=onesc[:])
        onesr_sb = cp.tile([1, 128], F32R, tag="onesr_sb")
        nc.sync.dma_start(out=onesr_sb[:], in_=onesr[:])
        bq_sb = cp.tile([128, MC], F32, tag="bq_sb")
        bk_sb = cp.tile([128, MC], F32, tag="bk_sb")
        bv_sb = cp.tile([128, MC], F32, tag="bv_sb")
        nc.sync.dma_start(out=bq_sb[:], in_=bq[:])
        nc.sync.dma_start(out=bk_sb[:], in_=bk[:])
        nc.sync.dma_start(out=bv_sb[:], in_=bv[:])
        m_sb = cp.tile([128, NT], F32, tag="m_sb")
        nc.sync.dma_start(out=m_sb[:], in_=mvec[:])
        cv_sb = cp.tile([128, 1], F32, tag="cv_sb")
        nc.sync.dma_start(out=cv_sb[:], in_=cvec[:])

        # persistent intermediates
        qT_sb = [cp.tile([128, S], F32R, tag=f"qT{j}", name=f"qT{j}") for j in range(4)]
        rq_all = cp.tile([128, 8 * NT], F32, tag="rq_all")
        kv_sb = cp.tile([128, 256], F32R, tag="kv_sb")
        ksum_sb = cp.tile([1, MC], F32R, tag="ksum_sb")
        vsum_sb = cp.tile([1, MC], F32R, tag="vsum_sb")
        ksumT_sb = cp.tile([128, 4], F32R, tag="ksumT_sb")
        ksumT3 = cp.tile([128, 8], F32R, tag="ksumT3")
        hssum_sb = cp.tile([128, NDM], F32, tag="hssum_sb")
        hssum_r = cp.tile([128, NDM], F32R, tag="hssum_r")
        ksum_bc = cp.tile([128, MC], F32, tag="ksum_bc")
        vsum_bc = cp.tile([128, MC], F32, tag="vsum_bc")

        # junk templates for cap_waits
        junk_ps = jpp.tile([1, 2], F32, tag="junk", bufs=1)
        nc.tensor.matmul(junk_ps[:], id_sb[:, 0:1], id_sb[:, 0:2], start=True, stop=True)
        junk_sb = cp.tile([1, 8], F32, tag="junk_sb")
        nc.sync.dma_start(out=junk_sb[0:1, 2:3], in_=junk_sb[0:1, 0:1])
        nc.vector.memset(junk_sb[0:1, 4:5], 0.0)
        nc.scalar.copy(junk_sb[0:1, 6:7], junk_sb[0:1, 4:5])
        nc.gpsimd.memset(junk_sb[0:1, 7:8], 0.0)

        # ======== PASS 1: projections, k-normalize, reductions, qT ========
        with tc.tile_pool(name="ps1", space="PSUM", bufs=1) as ps1:
            kv_ps = ps1.tile([128, MC], F32, tag="kv", bufs=1)
            ksum_ps = ps1.tile([1, MC], F32, tag="ksum", bufs=1)

            for ch in range(NCH):
                if ch == 0:
                    hsT_c = hsT_c0
                else:
                    hsT_c = iop.tile([128, NDM * 512], F32R, tag="hsT_c")
                    nc.sync.dma_start(
                        out=hsT_c[:].rearrange("p (j t) -> p j t", j=NDM),
                        in_=hsT[:, ch * 512:(ch + 1) * 512].rearrange(
                            "(j p) t -> p j t", p=128),
                    )
                hs_part = wkp.tile([128, NDM], F32, tag="hs_part", bufs=2)
                nc.vector.tensor_reduce(
                    hs_part[:], hsT_c[:].rearrange("p (j t) -> p j t", j=NDM),
                    axis=mybir.AxisListType.X, op=ADD)
                if ch == 0:
                    nc.vector.tensor_copy(hssum_sb[:], hs_part[:])
                else:
                    nc.vector.tensor_tensor(hssum_sb[:], hssum_sb[:], hs_part[:], ADD)
                for j4 in range(4):
                    si = ch * 4 + j4
                    pq = ps1.tile([128, MC], F32, tag="proj", bufs=3)
                    pk = ps1.tile([128, MC], F32, tag="proj", bufs=3)
                    pv = ps1.tile([128, MC], F32, tag="proj", bufs=3)
                    def blk_(dm):
                        return hsT_c[:, dm * 512 + j4 * 128: dm * 512 + (j4 + 1) * 128]
                    for dm in range(NDM):
                        nc.tensor.matmul(pk[:], blk_(dm), wk_sb[:, dm * MC:(dm + 1) * MC],
                                         start=(dm == 0), stop=(dm == NDM - 1))
                    for dm in range(NDM):
                        nc.tensor.matmul(pv[:], blk_(dm), wv_sb[:, dm * MC:(dm + 1) * MC],
                                         start=(dm == 0), stop=(dm == NDM - 1))
                    for dm in range(NDM):
                        nc.tensor.matmul(pq[:], blk_(dm), wq_sb[:, dm * MC:(dm + 1) * MC],
                                         start=(dm == 0), stop=(dm == NDM - 1))

                    m_ap = m_sb[:, si:si + 1]
                    # q = (pq + bq) * m   -> F32R
                    scr_q = wkp.tile([128, MC], F32, tag="scr_q", bufs=1)
                    nc.vector.tensor_tensor(scr_q[:], pq[:], bq_sb[:], ADD)
                    q_r = wkp.tile([128, MC], F32R, tag="q_r")
                    nc.scalar.activation(q_r[:], scr_q[:],
                                         mybir.ActivationFunctionType.Copy,
                                         scale=m_ap)
                    # k = (pk + bk) * m   -> F32R
                    scr_k = wkp.tile([128, MC], F32, tag="scr_k", bufs=1)
                    nc.vector.tensor_tensor(scr_k[:], pk[:], bk_sb[:], ADD)
                    k_r = wkp.tile([128, MC], F32R, tag="k_r", bufs=1)
                    nc.scalar.activation(k_r[:], scr_k[:],
                                         mybir.ActivationFunctionType.Copy,
                                         scale=m_ap)
                    # v = pv + bv  -> F32R
                    v_r = wkp.tile([128, MC], F32R, tag="v_r")
                    nc.vector.tensor_tensor(v_r[:], pv[:], bv_sb[:], ADD)

                    # rq = 1/(||q|| + EPS) per (token, head)
                    sq = wkp.tile([128, MC], F32, tag="sq", bufs=1)
                    nc.scalar.square(sq[:], q_r[:])
                    s2 = wkp.tile([128, 8], F32, tag="s2")
                    nc.vector.tensor_reduce(
                        s2[:], sq[:].rearrange("p (g d) -> p g d", g=8),
                        axis=mybir.AxisListType.X, op=ADD)
                    nn_ = wkp.tile([128, 8], F32, tag="nn_")
                    nc.scalar.sqrt(nn_[:], s2[:])
                    nc.vector.tensor_scalar_add(nn_[:], nn_[:], EPS)
                    nc.vector.reciprocal(rq_all[:, si * 8:(si + 1) * 8], nn_[:])

                    # k-hat = k / (||k|| + EPS)
                    sqk = wkp.tile([128, MC], F32, tag="sq", bufs=1)
                    nc.scalar.square(sqk[:], k_r[:])
                    s2k = wkp.tile([128, 8], F32, tag="s2")
                    nc.vector.tensor_reduce(
                        s2k[:], sqk[:].rearrange("p (g d) -> p g d", g=8),
                        axis=mybir.AxisListType.X, op=ADD)
                    nk = wkp.tile([128, 8], F32, tag="nn_")
                    nc.scalar.sqrt(nk[:], s2k[:])
                    nc.vector.tensor_scalar_add(nk[:], nk[:], EPS)
                    rg = wkp.tile([128, 8], F32, tag="rg")
                    nc.vector.reciprocal(rg[:], nk[:])
                    kh_r = wkp.tile([128, MC], F32R, tag="kh_r")
                    nc.vector.tensor_tensor(
                        kh_r[:].rearrange("p (g d) -> p g d", g=8),
                        k_r[:].rearrange("p (g d) -> p g d", g=8),
                        rg[:].unsqueeze(2).broadcast_to([128, 8, 64]), MULT)

                    # kv accumulation: per pair p, [128(2h), 128(2h e)] block
                    for p in range(4):
                        nc.tensor.matmul(
                            kv_ps[:, p * 128:(p + 1) * 128],
                            kh_r[:, p * 128:(p + 1) * 128],
                            v_r[:, p * 128:(p + 1) * 128],
                            start=(si == 0 and p == 0),
                            stop=(si == NT - 1 and p == 3),
                            skip_group_check=True)
                    # ksum / vsum
                    nc.tensor.matmul(ksum_ps[:], onesc_sb[:], kh_r[:],
                                     start=(si == 0), stop=(si == NT - 1),
                                     skip_group_check=True)

                    # qT via PE transpose
                    for j in range(4):
                        tq = ps1.tile([128, 128], F32R, tag="tq", bufs=2)
                        nc.tensor.transpose(tq[:], q_r[:, j * 128:(j + 1) * 128], id_sb[:])
                        nc.scalar.copy(
                            qT_sb[j][:, si * 128:(si + 1) * 128], tq[:])

            # ---- epilogue of pass 1 ----
            # kv diagonal blocks -> kv_sb [128, 256]
            for p in range(4):
                nc.vector.tensor_copy(
                    kv_sb[0:64, p * 64:(p + 1) * 64],
                    kv_ps[0:64, p * 128:p * 128 + 64])
                nc.vector.tensor_copy(
                    kv_sb[64:128, p * 64:(p + 1) * 64],
                    kv_ps[64:128, p * 128 + 64:(p + 1) * 128])
            nc.vector.tensor_copy(ksum_sb[:], ksum_ps[:])
            nc.vector.tensor_copy(hssum_r[:], hssum_sb[:])
            vsum_ps = ps1.tile([1, MC], F32, tag="proj", bufs=3)
            for dm in range(NDM):
                nc.tensor.matmul(vsum_ps[:], hssum_r[:, dm:dm + 1],
                                 wv_sb[:, dm * MC:(dm + 1) * MC],
                                 start=(dm == 0), stop=(dm == NDM - 1))
            nc.vector.tensor_copy(vsum_sb[:], vsum_ps[:])
            # ksumT [128, 4] via tiny transposes (N=2 with junk col)
            for j in range(4):
                tk = ps1.tile([128, 2], F32R, tag="tq", bufs=2)
                nc.tensor.transpose(
                    tk[:], ksum_sb[0:1, j * 128:(j + 1) * 128], id_sb[0:1, 0:2])
                nc.vector.tensor_copy(ksumT_sb[:, j:j + 1], tk[:, 0:1])
            # ksumT3 [128, 8]: per pair p, col 2p = ksum_{2p} on rows 0-63,
            # col 2p+1 = ksum_{2p+1} on rows 64-127, zeros elsewhere
            nc.sync.dma_start(out=ksumT3[:], in_=zer8[:])
            for p in range(4):
                nc.vector.tensor_copy(ksumT3[0:64, 2 * p:2 * p + 1],
                                      ksumT_sb[0:64, p:p + 1])
                nc.vector.tensor_copy(ksumT3[64:128, 2 * p + 1:2 * p + 2],
                                      ksumT_sb[64:128, p:p + 1])
            # partition-broadcast of ksum / vsum via K=1 matmuls
            pbc = ps1.tile([128, MC], F32, tag="proj", bufs=3)
            nc.tensor.matmul(pbc[:], onesr_sb[:], ksum_sb[:], start=True, stop=True)
            nc.vector.tensor_copy(ksum_bc[:], pbc[:])
            pbc2 = ps1.tile([128, MC], F32, tag="proj", bufs=3)
            nc.tensor.matmul(pbc2[:], onesr_sb[:], vsum_sb[:], start=True, stop=True)
            nc.vector.scalar_tensor_tensor(vsum_bc[:], bv_sb[:], float(S), pbc2[:],
                                           MULT, ADD)

        # ======== PASS 2: ctxT = kv^T qT, u, re-transpose, normalize ========
        with tc.tile_pool(name="ps2", space="PSUM", bufs=1) as ps2:
            for ch in range(NCH):
                cs = slice(ch * 512, (ch + 1) * 512)
                ctx_nat = [wkp.tile([128, MC], F32, tag=f"cn{j}", bufs=1, name=f"cn{j}")
                           for j in range(4)]
                u_nat = [wkp.tile([128, 8], F32, tag=f"un{j}", bufs=2, name=f"un{j}")
                         for j in range(4)]
                for p in range(4):
                    psA = ps2.tile([64, MC], F32, tag="ct", bufs=3)
                    psB = ps2.tile([64, MC], F32, tag="ct", bufs=3)
                    nc.tensor.matmul(psA[:], kv_sb[0:64, p * 64:(p + 1) * 64],
                                     qT_sb[p][0:64, cs], start=True, stop=True,
                                     tile_position=(0, 0))
                    nc.tensor.matmul(psB[:], kv_sb[64:128, p * 64:(p + 1) * 64],
                                     qT_sb[p][64:128, cs], start=True, stop=True,
                                     tile_position=(64, 0))
                    ctxT_sb = wkp.tile([128, MC], F32R, tag="ctxT_sb")
                    nc.scalar.copy(ctxT_sb[0:64, :], psA[:])
                    nc.scalar.copy(ctxT_sb[64:128, :], psB[:])
                    for j in range(4):
                        rt = ps2.tile([128, 128], F32R, tag="rt", bufs=2)
                        nc.tensor.transpose(
                            rt[:], ctxT_sb[:, j * 128:(j + 1) * 128], id_sb[:])
                        nc.scalar.copy(
                            ctx_nat[j][:, p * 128:(p + 1) * 128], rt[:])
                for j in range(4):
                    si = ch * 4 + j
                    pu = ps2.tile([128, 8], F32, tag="u", bufs=2)
                    for p in range(4):
                        nc.tensor.matmul(
                            pu[:, 2 * p:2 * p + 2],
                            qT_sb[p][:, si * 128:(si + 1) * 128],
                            ksumT3[:, 2 * p:2 * p + 2],
                            start=(p == 0), stop=(p == 3),
                            skip_group_check=True)
                    nc.vector.tensor_copy(u_nat[j][:], pu[:])
                # normalize + store
                o_ch = wkp.tile([128, 4 * MC], F32, tag="o_ch", bufs=1)
                for j in range(4):
                    si = ch * 4 + j
                    rq = rq_all[:, si * 8:(si + 1) * 8]
                    onum = wkp.tile([128, MC], F32, tag="onum", bufs=1)
                    nc.vector.tensor_tensor(
                        onum[:].rearrange("p (g d) -> p g d", g=8),
                        ctx_nat[j][:].rearrange("p (g d) -> p g d", g=8),
                        rq.unsqueeze(2).broadcast_to([128, 8, 64]), MULT)
                    nc.vector.tensor_tensor(onum[:], onum[:], vsum_bc[:], ADD)
                    dn = wkp.tile([128, 8], F32, tag="dn")
                    nc.vector.tensor_tensor(dn[:], u_nat[j][:], rq, MULT)
                    nc.vector.tensor_scalar_add(dn[:], dn[:], cv_sb[:, 0:1])
                    dn2 = wkp.tile([128, 8], F32, tag="dn2")
                    nc.vector.reciprocal(dn2[:], dn[:])
                    nc.vector.tensor_tensor(
                        o_ch[:, j * MC:(j + 1) * MC].rearrange("p (g d) -> p g d", g=8),
                        onum[:].rearrange("p (g d) -> p g d", g=8),
                        dn2[:].unsqueeze(2).broadcast_to([128, 8, 64]), MULT)
                nc.sync.dma_start(
                    out=out[ch * 512:(ch + 1) * 512, :].rearrange(
                        "(j p) m -> p j m", p=128),
                    in_=o_ch[:].rearrange("p (j m) -> p j m", j=4))

    _cap_waits(nc, wc_sem)
    return nc


def _cap_waits(nc, wc_sem):
    """trn2 walrus codegen allows only 1 sync wait per engine instruction.
    Move excess waits onto cloned junk instructions (same engine) inserted
    immediately before, each updating a dedicated junk semaphore."""
    f = nc.m.functions[0]
    tmpl = {}
    for b in f.blocks:
        for i in b.instructions:
            tn = type(i).__name__
            if tn == "InstMatmult" and "PE" not in tmpl and "junk" in str(i.outs[0]):
                tmpl["PE"] = i
            elif tn == "InstDMACopy" and "DMA" not in tmpl and "junk_sb" in str(i.outs[0]):
                tmpl["DMA"] = i
            elif tn == "InstMemset" and "junk_sb" in str(i.outs[0]):
                if i.engine == mybir.EngineType.DVE and "DVE" not in tmpl:
                    tmpl["DVE"] = i
                elif i.engine == mybir.EngineType.Pool and "POOL" not in tmpl:
                    tmpl["POOL"] = i
            elif tn == "InstActivation" and "ACT" not in tmpl and "junk_sb" in str(i.outs[0]):
                tmpl["ACT"] = i
    n = 0

    def make_carrier(kind, w):
        nonlocal n
        n += 1
        t = tmpl[kind]
        if kind == "PE":
            c = mybir.InstMatmult(
                name=f"Iwc-{n}",
                replication_resolution=0, replication_shift_amnt=0,
                replication_num_rows=0,
                start_tensor_calc=True, stop_tensor_calc=True,
                ins=list(t.ins), outs=list(t.outs),
                tile_size=t.tile_size, tile_position=t.tile_position,
            )
        elif kind == "DMA":
            c = mybir.InstDMACopy(
                name=f"Iwc-{n}", queue=t.queue, mode="Copy",
                ins=list(t.ins), outs=list(t.outs),
            )
        elif kind in ("DVE", "POOL"):
            c = mybir.InstMemset(
                name=f"Iwc-{n}", mode=t.mode, ins=[], outs=list(t.outs), constant=0,
            )
        else:
            c = mybir.InstActivation(
                name=f"Iwc-{n}", func=t.func, ins=list(t.ins), outs=list(t.outs),
            )
        c.engine = t.engine
        upd = bass_rust.SyncUpdate(
            sync_type="semaphore", id=wc_sem.num, ant_name=wc_sem.name,
            update_mode="sem-inc", update_value=1,
        )
        c.sync_info = bass_rust.SyncInfo(on_wait=[w], on_update=[upd])
        return c

    def kind_of(ins):
        eng = ins.engine
        if eng == mybir.EngineType.PE:
            return "PE"
        if eng == mybir.EngineType.SP:
            return "DMA"
        if eng == mybir.EngineType.DVE:
            return "DVE"
        if eng == mybir.EngineType.Activation:
            return "ACT"
        if eng == mybir.EngineType.Pool:
            return "POOL"
        return None

    for b in f.blocks:
        insts = b.instructions
        k = 0
        while k < len(insts):
            ins = insts[k]
            if not ins.name.startswith("Iwc"):
                si = ins.sync_info
                if si is not None and len(si.on_wait) > 1:
                    kind = kind_of(ins)
                    if kind is not None and kind in tmpl:
                        waits = list(si.on_wait)
                        for j, w in enumerate(waits[:-1]):
                            insts.insert(k + j, make_carrier(kind, w))
                        k += len(waits) - 1
                        ins.sync_info.on_wait = waits[-1:]
            k += 1


_CACHE = {}


def _get_nc():
    if "nc" not in _CACHE:
        _CACHE["nc"] = _build()
    return _CACHE["nc"]


def kernel(**inputs):
    hs = np.ascontiguousarray(np.asarray(inputs["hidden_states"], dtype=np.float32))
    mask = np.asarray(inputs["attention_mask"], dtype=np.float32)
    Wq = np.asarray(inputs["Wq"], dtype=np.float32)
    Wk = np.asarray(inputs["Wk"], dtype=np.float32)
    Wv = np.asarray(inputs["Wv"], dtype=np.float32)
    bq = np.asarray(inputs["bq"], dtype=np.float32)
    bk = np.asarray(inputs["bk"], dtype=np.float32)
    bv = np.asarray(inputs["bv"], dtype=np.float32)

    nc = _get_nc()
    ident = np.eye(128, dtype=np.float32)
    onesc = np.ones((128, 1), dtype=np.float32)
    onesr = np.ones((1, 128), dtype=np.float32)

    in_maps = []
    for core in range(8):
        b = core // 2
        hg = core % 2
        cols = slice(hg * MC, (hg + 1) * MC)
        m_s = mask[b, 0, 0, :] / 10000.0 + 1.0           # [S]
        c_const = EPS + float(m_s.sum())
        in_maps.append({
            "hsT": np.ascontiguousarray(hs[b].T),
            "wq": np.ascontiguousarray(Wq[:, cols]),
            "wk": np.ascontiguousarray(Wk[:, cols]),
            "wv": np.ascontiguousarray(Wv[:, cols]),
            "identr": ident,
            "onesc": onesc,
            "onesr": onesr,
            "bq": np.broadcast_to(bq[cols], (128, MC)).copy(),
            "bk": np.broadcast_to(bk[cols], (128, MC)).copy(),
            "bv": np.broadcast_to(bv[cols], (128, MC)).copy(),
            "mvec": np.ascontiguousarray(m_s.reshape(NT, 128).T),
            "cvec": np.full((128, 1), c_const, dtype=np.float32),
            "zer8": np.zeros((128, 8), dtype=np.float32),
        })

    res = run_bass_kernel_spmd(nc, in_maps, list(range(8)))
    out_full = np.empty((B, S, DM), dtype=np.float32)
    for core in range(8):
        b = core // 2
        hg = core % 2
        out_full[b, :, hg * MC:(hg + 1) * MC] = res.results[core]["out"]
    return out_full
